# revision 33
# baseline (speedup 1.0000x reference)
"""Trainium2 Bass kernel for a steerable group-CNN (GCNN) forward pass.

Pipeline (per image):
  conv1: 1->128 ch, 3x3, pad 1   (rotated-kernel construction done on host)
  relu
  conv2: 128->256 ch, 3x3, pad 1 (circulant group weight, built on host)
  relu
  group-pool: mean over inner-8 channel factor -> 32 ch
  fc: (32*28*28) -> 10

Device strategy (pure data parallel, batch 512 / 8 cores = 64 images/core):
  - conv1 as a single K=9 matmul per half image (im2col of x built on host):
      out[oc, pix] = sum_tap w1c[tap, oc] * x9[tap, pix]
    -> h laid out channels-on-partitions, pixels-free.  ReLU'd into a
    zero-padded 30x30 SBUF image (hpad) so conv2 can read shifted windows.
  - conv2 with *shifted activations as the stationary operand*:
      out[(y,x), oc] += hpad[:, y+dy, x+dx].T @ wt[:, (dy,dx), :]
    9 accumulating matmuls per 4-row chunk (M=112 pixels, N=256 channels).
    Output lands pixels-on-partitions / channels-free, which makes the
    group-pool a free-dim strided reduce on VectorE.
  - pool+fc folded: p = reduce_add over inner-8 channels; the 1/8 mean and
    the flatten order are folded into a host-rearranged fc weight. FC is a
    pixel-contraction matmul accumulated over (chunk, group) into one PSUM.
"""

import os

import numpy as np

import concourse.tile as tile
from concourse import bacc, mybir
from concourse.bass_utils import run_bass_kernel_spmd

G = 8
KS = 3
HW = 28
PW = HW + 2          # padded image width
NPIX = HW * HW       # 784
NCH1 = 128           # conv1 out channels (G*16)
NCH2 = 256           # conv2 out channels (G*32)
NA = 32              # pooled channels
NCLS = 10
# conv2 processes M=128 contiguous *padded* (30-wide) flat positions per chunk:
# chunk c covers padded-flat positions f = 128c + m.  Valid output pixels are
# f = y*30+x with x<28, y<28; everything else is a junk partition annihilated
# by zero rows in the host-built fc weight.
PIXM = 128
NCHK = -(-(HW * PW - 2) // PIXM)   # 7 chunks cover flat [0, 838)
# taps read hp[128c + m + dy*30 + dx]; max = 128*6 + 127 + 62 = 957
HP_LEN = 960
N_CORES = 8
B_TOT = 512
B_LOC = B_TOT // N_CORES      # 64
C_IMG = 8                     # images per x9 DMA chunk

# matmul operand dtype for conv2 (h storage + group weight): "f32", "f32r", "bf16"
CONV_DT = os.environ.get("GCNN_CONV_DT", "bf16")
# conv1 operand dtype (x im2col + rotated base weight).  f32r moving operands
# stream at 2 cycles/column (4-byte reads), so bf16 halves conv1 PE time and
# the x9 DMA bytes; end-to-end rel_fro stays ~4e-3.
C1_DT = os.environ.get("GCNN_C1_DT", "bf16")
# fc/pool dtype: "f32", "f32r", or "bf16".  f32r is fastest here: the fc is
# weight-load-bound (224 tiny-M matmuls), and the fp32 weight path pipelines
# loads at ~57ns/MM while bf16's separate LDWEIGHTS (col_grp-restricted,
# ~104ns, unhidden behind a 27ns matmul) serializes at ~110ns/MM.
FC_DT = os.environ.get("GCNN_FC_DT", "f32r")

_F32 = mybir.dt.float32
_BF16 = mybir.dt.bfloat16
_F32R = mybir.dt.float32r


def _store_dt(kind):
    if kind == "bf16":
        return _BF16
    if kind == "f32r":
        return _F32R
    return _F32


def _np_dt(kind):
    import ml_dtypes
    return ml_dtypes.bfloat16 if kind == "bf16" else np.float32


def _mm(ap, kind):
    return ap


# ---------------------------------------------------------------------------
# Host-side weight construction (replicates the reference's jax math in numpy)
# ---------------------------------------------------------------------------

def _bilinear_sample(img, px, py):
    K = img.shape[-1]
    x0 = np.floor(px)
    y0 = np.floor(py)
    wx = (px - x0).astype(np.float32)
    wy = (py - y0).astype(np.float32)
    x0i = x0.astype(np.int32)
    y0i = y0.astype(np.int32)

    def gather(yi, xi):
        valid = (yi >= 0) & (yi < K) & (xi >= 0) & (xi < K)
        yc = np.clip(yi, 0, K - 1)
        xc = np.clip(xi, 0, K - 1)
        return img[:, :, yc, xc] * valid.astype(img.dtype)

    return (gather(y0i, x0i) * (1 - wx) * (1 - wy)
            + gather(y0i, x0i + 1) * wx * (1 - wy)
            + gather(y0i + 1, x0i) * (1 - wx) * wy
            + gather(y0i + 1, x0i + 1) * wx * wy)


def _rotated_kernels(base, group_order):
    K = base.shape[-1]
    coords = ((2.0 * np.arange(K, dtype=np.float32) + 1.0) / K - 1.0).astype(np.float32)
    xs, ys = np.meshgrid(coords, coords, indexing="xy")
    out = np.empty((group_order,) + base.shape, np.float32)
    for k in range(group_order):
        theta = np.float32(2.0 * np.pi * k / group_order)
        c, s = np.float32(np.cos(theta)), np.float32(np.sin(theta))
        gx = c * xs - s * ys
        gy = s * xs + c * ys
        px = ((gx + 1.0) * K - 1.0) / 2.0
        py = ((gy + 1.0) * K - 1.0) / 2.0
        out[k] = _bilinear_sample(base, px.astype(np.float32), py.astype(np.float32))
    return out


def _host_prep(x, base_weight, w2, fc_w, fc_b):
    conv_np = _np_dt(CONV_DT)
    c1_np = _np_dt("f32r" if C1_DT == "mixed" else C1_DT)
    w1_np = _np_dt("bf16" if C1_DT == "mixed" else C1_DT)
    fc_np = _np_dt(FC_DT)

    rk = _rotated_kernels(base_weight.astype(np.float32), G)   # (G, 16, 1, 3, 3)
    w1 = rk.reshape(G * 16, 1, KS, KS)                         # (128, 1, 3, 3)
    w1c = np.ascontiguousarray(w1[:, 0].reshape(NCH1, 9).T)    # (9, 128), tap=dy*3+dx

    gi = np.arange(G)[:, None]
    hi = np.arange(G)[None, :]
    idx = (gi - hi) % G
    Wc = w2[:, :, idx]                                          # (32, 16, G, G, 3, 3)
    Wbig = np.transpose(Wc, (2, 0, 1, 3, 4, 5)).reshape(NCH2, NCH1, KS, KS)
    # wt[ic, tap, oc] = Wbig[oc, ic, dy, dx]
    wt = np.ascontiguousarray(np.transpose(Wbig, (1, 2, 3, 0))).reshape(NCH1, 9 * NCH2)

    # fcw[m, c, a*10+n] = fc_w[n, a*784 + y*28 + x] / 8 for f = 128c+m = y*30+x
    # when (y, x) is a real pixel; zero for junk positions.
    f8 = (fc_w.astype(np.float64) / 8.0).astype(np.float32).reshape(NCLS, NA, HW, HW)
    fcw = np.zeros((PIXM, NCHK, NA, NCLS), np.float32)
    for c in range(NCHK):
        for m in range(PIXM):
            ff = c * PIXM + m
            yy, xx = ff // PW, ff % PW
            if yy < HW and xx < HW:
                fcw[m, c] = f8[:, :, yy, xx].T
    fcw = np.ascontiguousarray(fcw.reshape(PIXM, NCHK * NA * NCLS))

    # im2col of padded x: x9[tap, b, pix] = xpad[b, y+dy, x+dx]
    B = x.shape[0]
    xp = np.zeros((B, PW, PW), np.float32)
    xp[:, 1:1 + HW, 1:1 + HW] = x[:, 0]
    x9 = np.empty((9, B, HW, HW), np.float32)
    for dy in range(3):
        for dx in range(3):
            x9[dy * 3 + dx] = xp[:, dy:dy + HW, dx:dx + HW]
    x9 = x9.reshape(9, B, NPIX)

    return {
        "x9": np.ascontiguousarray(x9.astype(c1_np)),
        "w1c": np.ascontiguousarray(w1c.astype(w1_np)),
        "wt": np.ascontiguousarray(wt.astype(conv_np)),
        "fcw": np.ascontiguousarray(fcw.astype(fc_np)),
        "fcb": np.ascontiguousarray(fc_b.reshape(1, NCLS).astype(np.float32)),
    }


# ---------------------------------------------------------------------------
# Device kernel
# ---------------------------------------------------------------------------

def build_bass():
    from contextlib import ExitStack

    conv_sdt = _store_dt(CONV_DT)
    c1_sdt = _store_dt("f32r" if C1_DT == "mixed" else C1_DT)
    w1_sdt = _store_dt("bf16" if C1_DT == "mixed" else C1_DT)
    fc_sdt = _store_dt(FC_DT)

    nc = bacc.Bacc()
    x9_d = nc.declare_dram_parameter("x9", [9, B_LOC, NPIX], c1_sdt, isOutput=False)
    w1c_d = nc.declare_dram_parameter("w1c", [9, NCH1], w1_sdt, isOutput=False)
    wt_d = nc.declare_dram_parameter("wt", [NCH1, 9 * NCH2], conv_sdt, isOutput=False)
    fcw_d = nc.declare_dram_parameter("fcw", [PIXM, NCHK * NA * NCLS], fc_sdt,
                                      isOutput=False)
    fcb_d = nc.declare_dram_parameter("fcb", [1, NCLS], _F32, isOutput=False)
    # out is [classes, batch]; the host transposes.  A [batch, classes] DRAM
    # layout would need a `b n -> n b` rearranged DMA = 640 strided 4-byte
    # descriptors (~8 us measured); this way it's 10 contiguous rows.
    out_d = nc.declare_dram_parameter("out", [NCLS, B_LOC], _F32, isOutput=True)

    with tile.TileContext(nc) as tc, ExitStack() as ctx:
        consts = ctx.enter_context(tc.tile_pool(name="consts", bufs=1))
        x9_pool = ctx.enter_context(tc.tile_pool(name="x9", bufs=2))
        hp_pool = ctx.enter_context(tc.tile_pool(name="hpad", bufs=7))
        h2_pool = ctx.enter_context(tc.tile_pool(name="h2r", bufs=4))
        # ps1=4 so a 2-image conv1 burst (4 matmuls) never waits on relu1;
        # ps2=3 is enough because relu2+pool (DVE) trail a chunk-pair by far
        # less than 3 pair-periods.  4+3+1 = 8 PSUM banks.
        ps1_pool = ctx.enter_context(tc.tile_pool(name="ps1", bufs=4, space="PSUM"))
        ps2_pool = ctx.enter_context(tc.tile_pool(name="ps2", bufs=3, space="PSUM"))
        psfc_pool = ctx.enter_context(tc.tile_pool(name="psfc", bufs=1, space="PSUM"))

        # First input chunk: tiny (2 images) and issued before everything else
        # so conv1 can start as early as possible.
        x9_first = consts.tile([9, 2, NPIX], c1_sdt)
        nc.sync.dma_start(x9_first[:], x9_d[:, 0:2, :])
        w1c_t = consts.tile([9, NCH1], w1_sdt)
        nc.sync.dma_start(w1c_t[:], w1c_d[:])

        # PE warm-up: dependency-free matmuls keep the tensor engine busy from
        # engine start, flipping the HAM clock gate to 2.4 GHz before the real
        # work arrives and hiding the initial weight/input DMA latency.  Uses
        # ps1-pool tiles so no PSUM bank is held for the rest of the kernel.
        warm_sb = consts.tile([NCH1, 512], conv_sdt)
        nc.vector.memset(warm_sb[:].bitcast(_F32) if conv_sdt == _F32R else warm_sb[:],
                         0.125)
        for _ in range(3):
            warm_ps = ps1_pool.tile([NCH1, half := NPIX // 2], _F32, tag="ps1")
            nc.tensor.matmul(warm_ps[:], lhsT=warm_sb[:, :NCH1],
                             rhs=warm_sb[:, :half], start=True, stop=True)
            nc.tensor.matmul(warm_ps[:], lhsT=warm_sb[:, :NCH1],
                             rhs=warm_sb[:, :half], start=True, stop=True)

        # The first two in-loop x9 chunks are issued BEFORE the big wt
        # transfer: DMA queues serialize, and the ~590KB wt load otherwise
        # delays chunk 2's landing past when conv1(2) needs it (~2us stall).
        x9_early = {}
        for xb, xsz in ((2, 2), (4, 4)):
            xt = x9_pool.tile([9, xsz, NPIX], c1_sdt, tag="x9")
            nc.sync.dma_start(xt[:], x9_d[:, xb:xb + xsz, :])
            x9_early[xb] = xt

        # resident tensors
        wt_t = consts.tile([NCH1, 9, NCH2], conv_sdt)
        nc.sync.dma_start(wt_t[:], wt_d[:].rearrange("p (t o) -> p t o", o=NCH2))
        # bias row [1, 10] and a ones row [1, 64] feed the K=1 bias matmul
        fcb_t = consts.tile([1, NCLS], _F32)
        nc.sync.dma_start(fcb_t[:], fcb_d[:])
        ones_t = consts.tile([1, B_LOC], _F32)
        nc.vector.memset(ones_t[:], 1.0)
        # fcw is only needed by the fc tail; load it off the critical start path
        # fc operands stay f32r: bf16 fc matmuls pay a serialized ~104ns
        # LDWEIGHTS per MM (col_grp-restricted), and mixed f32r/bf16 operands
        # are rejected by the compiler; the fp32 path sustains ~57ns/MM.
        fcw_t = consts.tile([PIXM, NCHK, NA * NCLS], fc_sdt)
        p_all = consts.tile([PIXM, NCHK, B_LOC, NA], fc_sdt)

        half = NPIX // 2  # 392

        def _ms(ap):
            # memset rejects float32r; zero bits through a float32 view
            nc.gpsimd.memset(ap.bitcast(_F32) if conv_sdt == _F32R else ap, 0.0)

        def conv1(b, x9_t, bi):
            """h(b) = relu(conv1(x(b))) written into a padded 30x30 image."""
            hp = hp_pool.tile([NCH1, HP_LEN], conv_sdt, tag="hp")
            hp3 = hp[:, :PW * PW].rearrange("p (y x) -> p y x", x=PW)
            # zero the 1-pixel border (interior is fully overwritten below)
            _ms(hp3[:, 0, :])
            _ms(hp3[:, PW - 1, :])
            _ms(hp3[:, 1:PW - 1, 0])
            _ms(hp3[:, 1:PW - 1, PW - 1])
            _ms(hp[:, PW * PW:])
            for h in range(2):
                ps1 = ps1_pool.tile([NCH1, half], _F32, tag="ps1")
                nc.tensor.matmul(
                    ps1[:],
                    lhsT=_mm(w1c_t[:], C1_DT),
                    rhs=_mm(x9_t[:, bi, h * half:(h + 1) * half], C1_DT),
                    start=True, stop=True,
                )
                # relu + downcast into hpad interior rows 14h..14h+13
                dst = hp3[:, 1 + 14 * h:1 + 14 * (h + 1), 1:1 + HW]
                nc.scalar.activation(
                    dst, ps1[:].rearrange("p (y x) -> p y x", x=HW),
                    mybir.ActivationFunctionType.Relu,
                )
            return hp

        def conv2(b, hp):
            """h2(b) -> relu -> group-pool into p_all[:, :, b, :].

            Chunks are processed in pairs sharing one PSUM bank (2 x 256 f32
            = one 2KB bank): the pair's first matmul start=True zeroes the
            whole bank; the second chunk's matmuls rely on pending-zero for
            their first write.  Halves the sem-inc rounds on PE and the
            relu/pool op counts."""
            for ci, cs in enumerate([(0, 1), (2, 3), (4, 5), (6,)]):
                nc2 = len(cs) * NCH2
                ps2 = ps2_pool.tile([PIXM, nc2], _F32, tag="ps2")
                nmm = len(cs) * 9
                i = 0
                for k, c in enumerate(cs):
                    for tap in range(9):
                        dy, dx = tap // 3, tap % 3
                        off = PIXM * c + dy * PW + dx
                        lhsT = hp[:, off:off + PIXM]
                        nc.tensor.matmul(
                            ps2[:, k * NCH2:(k + 1) * NCH2],
                            lhsT=_mm(lhsT, CONV_DT),
                            rhs=_mm(wt_t[:, tap, :], CONV_DT),
                            start=(i == 0), stop=(i == nmm - 1),
                        )
                        i += 1
                # relu2 on the vector engine (same engine as the pool, so no
                # cross-engine relay; measured faster than scalar or
                # alternating-engine variants).
                h2r = h2_pool.tile([PIXM, nc2], _BF16, tag="h2r")
                nc.vector.tensor_scalar_max(h2r[:], ps2[:], 0.0)
                with nc.allow_low_precision(reason="pool sum feeds reduced-precision fc"):
                    nc.vector.tensor_reduce(
                        p_all[:, cs[0]:cs[0] + len(cs), b, :],
                        h2r[:].rearrange("p (c a k) -> p c a k", k=G, a=NA),
                        axis=mybir.AxisListType.X,
                        op=mybir.AluOpType.add,
                    )

        # software-pipelined main loop (conv1 runs DEPTH images ahead of
        # conv2 so relu/memset latency never stalls the PE); images 0-1 come
        # from the early x9_first chunk
        # Software-pipelined main loop over image PAIRS: conv1 for two images
        # is emitted as one burst so the w1c LDWEIGHTS (row_grp=q0, conflicts
        # with in-flight full-array matmuls and pays a ~95ns pipeline-drain
        # bubble) is paid once per two images instead of once per image.
        # conv1 runs DEPTH=4 images ahead of conv2.
        DEPTH = 4
        hps = {}
        # staggered input chunks: small early chunks land before the compute
        # pipeline drains (a single 8-image first chunk measured a ~1.5us PE
        # stall at startup waiting on its ~multi-us DMA).
        sizes = [2, 2, 4] + [C_IMG] * ((B_LOC - 8) // C_IMG)
        assert sum(sizes) == B_LOC
        x9_t, x0, sz, ci = x9_first, 0, 2, 1
        for pb in range(0, B_LOC + DEPTH, 2):
            if pb < B_LOC:
                if pb == x0 + sz:
                    x0, sz = pb, sizes[ci]
                    ci += 1
                    if pb in x9_early:
                        x9_t = x9_early.pop(pb)
                    else:
                        x9_t = x9_pool.tile([9, sz, NPIX], c1_sdt, tag="x9")
                        nc.sync.dma_start(x9_t[:], x9_d[:, x0:x0 + sz, :])
                hps[pb] = conv1(pb, x9_t, pb - x0)
                hps[pb + 1] = conv1(pb + 1, x9_t, pb + 1 - x0)
            if pb >= DEPTH:
                conv2(pb - DEPTH, hps.pop(pb - DEPTH))
                conv2(pb - DEPTH + 1, hps.pop(pb - DEPTH + 1))

        nc.sync.dma_start(fcw_t[:], fcw_d[:].rearrange("p (c m) -> p c m", m=NA * NCLS))

        # fc: out[n, bb] = fcb[n] + sum_ca fcw[:, c, a*10+n].T @ p_all[:, c, :, a]
        # The bias enters as a K=1 rank-one matmul (fcb x ones) opening the
        # accumulation, and the result DMAs straight out of PSUM — no separate
        # vector bias-add or SBUF staging on the critical tail.
        fc_ps = psfc_pool.tile([NCLS, B_LOC], _F32)
        nc.tensor.matmul(fc_ps[:], lhsT=fcb_t[:], rhs=ones_t[:],
                         start=True, stop=False)
        nmm = NCHK * NA
        i = 0
        for c in range(NCHK):
            for a in range(NA):
                nc.tensor.matmul(
                    fc_ps[:],
                    lhsT=_mm(fcw_t[:, c, a * NCLS:(a + 1) * NCLS], FC_DT),
                    rhs=_mm(p_all[:, c, :, a], FC_DT),
                    start=False, stop=(i == nmm - 1),
                )
                i += 1
        # DMA cannot read PSUM; stage through SBUF on the scalar engine,
        # which is idle at the tail (vector is still draining pools).
        out_sb = consts.tile([NCLS, B_LOC], _F32)
        nc.scalar.activation(out_sb[:], fc_ps[:],
                             mybir.ActivationFunctionType.Copy)
        nc.sync.dma_start(out_d[:], out_sb[:])

    if not nc.is_finalized():
        nc.finalize()
    return nc


_NC_CACHE = {}


def _get_nc():
    key = (CONV_DT, FC_DT)
    if key not in _NC_CACHE:
        _NC_CACHE[key] = build_bass()
    return _NC_CACHE[key]


def _run(x, base_weight, w2, fc_w, fc_b, **spmd_kwargs):
    x = np.asarray(x, np.float32)
    base_weight = np.asarray(base_weight, np.float32)
    w2 = np.asarray(w2, np.float32)
    fc_w = np.asarray(fc_w, np.float32)
    fc_b = np.asarray(fc_b, np.float32)

    prep = _host_prep(x, base_weight, w2, fc_w, fc_b)
    nc = _get_nc()
    in_maps = []
    for i in range(N_CORES):
        m = dict(prep)
        m["x9"] = np.ascontiguousarray(prep["x9"][:, i * B_LOC:(i + 1) * B_LOC, :])
        in_maps.append(m)
    res = run_bass_kernel_spmd(nc, in_maps, list(range(N_CORES)), **spmd_kwargs)
    out = np.concatenate(
        [np.ascontiguousarray(res.results[i]["out"].T) for i in range(N_CORES)],
        axis=0)
    return out, res


def kernel(x, base_weight, w2, fc_w, fc_b):
    out, _ = _run(x, base_weight, w2, fc_w, fc_b)
    return out



# revision 43
# speedup vs baseline: 1.0123x; 1.0123x over previous
"""Trainium2 Bass kernel for a steerable group-CNN (GCNN) forward pass.

Pipeline (per image):
  conv1: 1->128 ch, 3x3, pad 1   (rotated-kernel construction done on host)
  relu
  conv2: 128->256 ch, 3x3, pad 1 (circulant group weight, built on host)
  relu
  group-pool: mean over inner-8 channel factor -> 32 ch
  fc: (32*28*28) -> 10

Device strategy (pure data parallel, batch 512 / 8 cores = 64 images/core):
  - conv1 as a single K=9 matmul per half image (im2col of x built on host):
      out[oc, pix] = sum_tap w1c[tap, oc] * x9[tap, pix]
    -> h laid out channels-on-partitions, pixels-free.  ReLU'd into a
    zero-padded 30x30 SBUF image (hpad) so conv2 can read shifted windows.
  - conv2 with *shifted activations as the stationary operand*:
      out[(y,x), oc] += hpad[:, y+dy, x+dx].T @ wt[:, (dy,dx), :]
    9 accumulating matmuls per 4-row chunk (M=112 pixels, N=256 channels).
    Output lands pixels-on-partitions / channels-free, which makes the
    group-pool a free-dim strided reduce on VectorE.
  - pool+fc folded: p = reduce_add over inner-8 channels; the 1/8 mean and
    the flatten order are folded into a host-rearranged fc weight. FC is a
    pixel-contraction matmul accumulated over (chunk, group) into one PSUM.
"""

import os

import numpy as np

import concourse.tile as tile
from concourse import bacc, mybir
from concourse.bass_utils import run_bass_kernel_spmd

G = 8
KS = 3
HW = 28
PW = HW + 2          # padded image width
NPIX = HW * HW       # 784
NCH1 = 128           # conv1 out channels (G*16)
NCH2 = 256           # conv2 out channels (G*32)
NA = 32              # pooled channels
NCLS = 10
# conv2 processes M=128 contiguous *padded* (30-wide) flat positions per chunk:
# chunk c covers padded-flat positions f = 128c + m.  Valid output pixels are
# f = y*30+x with x<28, y<28; everything else is a junk partition annihilated
# by zero rows in the host-built fc weight.
PIXM = 128
NCHK = -(-(HW * PW - 2) // PIXM)   # 7 chunks cover flat [0, 838)
# taps read hp[128c + m + dy*30 + dx]; max = 128*6 + 127 + 62 = 957
HP_LEN = 960
N_CORES = 8
B_TOT = 512
B_LOC = B_TOT // N_CORES      # 64
C_IMG = 8                     # images per x9 DMA chunk

# matmul operand dtype for conv2 (h storage + group weight): "f32", "f32r", "bf16"
CONV_DT = os.environ.get("GCNN_CONV_DT", "bf16")
# conv1 operand dtype (x im2col + rotated base weight).  f32r moving operands
# stream at 2 cycles/column (4-byte reads), so bf16 halves conv1 PE time and
# the x9 DMA bytes; end-to-end rel_fro stays ~4e-3.
C1_DT = os.environ.get("GCNN_C1_DT", "bf16")
# fc/pool dtype: "f32", "f32r", or "bf16".  f32r is fastest here: the fc is
# weight-load-bound (224 tiny-M matmuls), and the fp32 weight path pipelines
# loads at ~57ns/MM while bf16's separate LDWEIGHTS (col_grp-restricted,
# ~104ns, unhidden behind a 27ns matmul) serializes at ~110ns/MM.
FC_DT = os.environ.get("GCNN_FC_DT", "f32r")

_F32 = mybir.dt.float32
_BF16 = mybir.dt.bfloat16
_F32R = mybir.dt.float32r


def _store_dt(kind):
    if kind == "bf16":
        return _BF16
    if kind == "f32r":
        return _F32R
    return _F32


def _np_dt(kind):
    import ml_dtypes
    return ml_dtypes.bfloat16 if kind == "bf16" else np.float32


def _mm(ap, kind):
    return ap


# ---------------------------------------------------------------------------
# Host-side weight construction (replicates the reference's jax math in numpy)
# ---------------------------------------------------------------------------

def _bilinear_sample(img, px, py):
    K = img.shape[-1]
    x0 = np.floor(px)
    y0 = np.floor(py)
    wx = (px - x0).astype(np.float32)
    wy = (py - y0).astype(np.float32)
    x0i = x0.astype(np.int32)
    y0i = y0.astype(np.int32)

    def gather(yi, xi):
        valid = (yi >= 0) & (yi < K) & (xi >= 0) & (xi < K)
        yc = np.clip(yi, 0, K - 1)
        xc = np.clip(xi, 0, K - 1)
        return img[:, :, yc, xc] * valid.astype(img.dtype)

    return (gather(y0i, x0i) * (1 - wx) * (1 - wy)
            + gather(y0i, x0i + 1) * wx * (1 - wy)
            + gather(y0i + 1, x0i) * (1 - wx) * wy
            + gather(y0i + 1, x0i + 1) * wx * wy)


def _rotated_kernels(base, group_order):
    K = base.shape[-1]
    coords = ((2.0 * np.arange(K, dtype=np.float32) + 1.0) / K - 1.0).astype(np.float32)
    xs, ys = np.meshgrid(coords, coords, indexing="xy")
    out = np.empty((group_order,) + base.shape, np.float32)
    for k in range(group_order):
        theta = np.float32(2.0 * np.pi * k / group_order)
        c, s = np.float32(np.cos(theta)), np.float32(np.sin(theta))
        gx = c * xs - s * ys
        gy = s * xs + c * ys
        px = ((gx + 1.0) * K - 1.0) / 2.0
        py = ((gy + 1.0) * K - 1.0) / 2.0
        out[k] = _bilinear_sample(base, px.astype(np.float32), py.astype(np.float32))
    return out


def _host_prep(x, base_weight, w2, fc_w, fc_b):
    conv_np = _np_dt(CONV_DT)
    c1_np = _np_dt("f32r" if C1_DT == "mixed" else C1_DT)
    w1_np = _np_dt("bf16" if C1_DT == "mixed" else C1_DT)
    fc_np = _np_dt(FC_DT)

    rk = _rotated_kernels(base_weight.astype(np.float32), G)   # (G, 16, 1, 3, 3)
    w1 = rk.reshape(G * 16, 1, KS, KS)                         # (128, 1, 3, 3)
    # (128, 128): taps in rows 0-8, rows 9-127 zero.  The zero-padded K=128
    # keeps conv1's matmuls full-array (a K=9 stationary gets row_grp=q0 and
    # its LDWEIGHTS then conflicts with in-flight full-array matmuls, paying
    # a ~95ns pipeline-drain bubble at every conv1 group).
    w1c = np.zeros((NCH1, NCH1), np.float32)
    w1c[:9] = w1[:, 0].reshape(NCH1, 9).T                      # tap=dy*3+dx

    gi = np.arange(G)[:, None]
    hi = np.arange(G)[None, :]
    idx = (gi - hi) % G
    Wc = w2[:, :, idx]                                          # (32, 16, G, G, 3, 3)
    Wbig = np.transpose(Wc, (2, 0, 1, 3, 4, 5)).reshape(NCH2, NCH1, KS, KS)
    # wt[ic, tap, oc] = Wbig[oc, ic, dy, dx]
    wt = np.ascontiguousarray(np.transpose(Wbig, (1, 2, 3, 0))).reshape(NCH1, 9 * NCH2)

    # fcw[m, c, a*10+n] = fc_w[n, a*784 + y*28 + x] / 8 for f = 128c+m = y*30+x
    # when (y, x) is a real pixel; zero for junk positions.
    f8 = (fc_w.astype(np.float64) / 8.0).astype(np.float32).reshape(NCLS, NA, HW, HW)
    fcw = np.zeros((PIXM, NCHK, NA, NCLS), np.float32)
    for c in range(NCHK):
        for m in range(PIXM):
            ff = c * PIXM + m
            yy, xx = ff // PW, ff % PW
            if yy < HW and xx < HW:
                fcw[m, c] = f8[:, :, yy, xx].T
    fcw = np.ascontiguousarray(fcw.reshape(PIXM, NCHK * NA * NCLS))

    # im2col of padded x: x9[tap, b, pix] = xpad[b, y+dy, x+dx]
    B = x.shape[0]
    xp = np.zeros((B, PW, PW), np.float32)
    xp[:, 1:1 + HW, 1:1 + HW] = x[:, 0]
    x9 = np.empty((9, B, HW, HW), np.float32)
    for dy in range(3):
        for dx in range(3):
            x9[dy * 3 + dx] = xp[:, dy:dy + HW, dx:dx + HW]
    x9 = x9.reshape(9, B, NPIX)

    return {
        "x9": np.ascontiguousarray(x9.astype(c1_np)),
        "w1c": np.ascontiguousarray(w1c.astype(w1_np)),
        "wt": np.ascontiguousarray(wt.astype(conv_np)),
        "fcw": np.ascontiguousarray(fcw.astype(fc_np)),
        "fcb": np.ascontiguousarray(fc_b.reshape(1, NCLS).astype(np.float32)),
    }


# ---------------------------------------------------------------------------
# Device kernel
# ---------------------------------------------------------------------------

def build_bass():
    from contextlib import ExitStack

    conv_sdt = _store_dt(CONV_DT)
    c1_sdt = _store_dt("f32r" if C1_DT == "mixed" else C1_DT)
    w1_sdt = _store_dt("bf16" if C1_DT == "mixed" else C1_DT)
    fc_sdt = _store_dt(FC_DT)

    nc = bacc.Bacc()
    x9_d = nc.declare_dram_parameter("x9", [9, B_LOC, NPIX], c1_sdt, isOutput=False)
    w1c_d = nc.declare_dram_parameter("w1c", [NCH1, NCH1], w1_sdt, isOutput=False)
    wt_d = nc.declare_dram_parameter("wt", [NCH1, 9 * NCH2], conv_sdt, isOutput=False)
    fcw_d = nc.declare_dram_parameter("fcw", [PIXM, NCHK * NA * NCLS], fc_sdt,
                                      isOutput=False)
    fcb_d = nc.declare_dram_parameter("fcb", [1, NCLS], _F32, isOutput=False)
    # out is [classes, batch]; the host transposes.  A [batch, classes] DRAM
    # layout would need a `b n -> n b` rearranged DMA = 640 strided 4-byte
    # descriptors (~8 us measured); this way it's 10 contiguous rows.
    out_d = nc.declare_dram_parameter("out", [NCLS, B_LOC], _F32, isOutput=True)

    with tile.TileContext(nc) as tc, ExitStack() as ctx:
        consts = ctx.enter_context(tc.tile_pool(name="consts", bufs=1))
        x9_pool = ctx.enter_context(tc.tile_pool(name="x9", bufs=2))
        hp_pool = ctx.enter_context(tc.tile_pool(name="hpad", bufs=7))
        h2_pool = ctx.enter_context(tc.tile_pool(name="h2r", bufs=4))
        # ps1=4 so a 2-image conv1 burst (4 matmuls) never waits on relu1;
        # ps2=3 is enough because relu2+pool (DVE) trail a chunk-pair by far
        # less than 3 pair-periods.  4+3+1 = 8 PSUM banks.
        ps1_pool = ctx.enter_context(tc.tile_pool(name="ps1", bufs=4, space="PSUM"))
        ps2_pool = ctx.enter_context(tc.tile_pool(name="ps2", bufs=3, space="PSUM"))
        psfc_pool = ctx.enter_context(tc.tile_pool(name="psfc", bufs=1, space="PSUM"))

        # First input chunk: tiny (2 images) and issued before everything else
        # so conv1 can start as early as possible.  Early chunks stay
        # 9-partition (K=9 conv1, row_grp=q0 — bubbles only at startup);
        # steady-state chunks land in two fixed 128-partition buffers whose
        # rows 9-127 are zeroed once, so steady conv1 contracts K=128 with
        # full-array untagged matmuls (no LDWEIGHTS drain bubbles).
        x9_first = consts.tile([9, 2, NPIX], c1_sdt)
        nc.sync.dma_start(x9_first[:], x9_d[:, 0:2, :])
        w1c_t = consts.tile([NCH1, NCH1], w1_sdt)
        nc.sync.dma_start(w1c_t[:], w1c_d[:])

        # PE warm-up: dependency-free matmuls keep the tensor engine busy from
        # engine start, flipping the HAM clock gate to 2.4 GHz before the real
        # work arrives and hiding the initial weight/input DMA latency.  Uses
        # ps1-pool tiles so no PSUM bank is held for the rest of the kernel.
        warm_sb = consts.tile([NCH1, 512], conv_sdt)
        nc.vector.memset(warm_sb[:].bitcast(_F32) if conv_sdt == _F32R else warm_sb[:],
                         0.125)
        for _ in range(3):
            warm_ps = ps1_pool.tile([NCH1, half := NPIX // 2], _F32, tag="ps1")
            nc.tensor.matmul(warm_ps[:], lhsT=warm_sb[:, :NCH1],
                             rhs=warm_sb[:, :half], start=True, stop=True)
            nc.tensor.matmul(warm_ps[:], lhsT=warm_sb[:, :NCH1],
                             rhs=warm_sb[:, :half], start=True, stop=True)

        # The first two in-loop x9 chunks are issued BEFORE the big wt
        # transfer: DMA queues serialize, and the ~590KB wt load otherwise
        # delays chunk 2's landing past when conv1(2) needs it (~2us stall).
        # Early chunks are 9-partition tiles (K=9 conv1).
        x9_early = {}
        for xb, xsz in ((2, 2), (4, 4)):
            xt = x9_pool.tile([9, xsz, NPIX], c1_sdt, tag="x9")
            nc.sync.dma_start(xt[:], x9_d[:, xb:xb + xsz, :])
            x9_early[xb] = xt

        # Two fixed 128-partition buffers for the steady 8-image chunks;
        # their memsets (rows 9-127 stay zero forever) are emitted inside the
        # loop at pb==2 so the gpsimd queue serves hp(0..3) borders first.
        x9_big = [consts.tile([NCH1, C_IMG, NPIX], c1_sdt, name=f"x9big{j}")
                  for j in range(2)]

        def x9_load(xb, xsz, bi):
            xt = x9_big[bi % 2]
            nc.sync.dma_start(xt[0:9, 0:xsz], x9_d[:, xb:xb + xsz, :])
            return xt

        # resident tensors
        wt_t = consts.tile([NCH1, 9, NCH2], conv_sdt)
        nc.sync.dma_start(wt_t[:], wt_d[:].rearrange("p (t o) -> p t o", o=NCH2))
        # bias row [1, 10] and a ones row [1, 64] feed the K=1 bias matmul
        fcb_t = consts.tile([1, NCLS], _F32)
        nc.sync.dma_start(fcb_t[:], fcb_d[:])
        ones_t = consts.tile([1, B_LOC], _F32)
        nc.vector.memset(ones_t[:], 1.0)
        # fcw is only needed by the fc tail; load it off the critical start path
        # fc operands stay f32r: bf16 fc matmuls pay a serialized ~104ns
        # LDWEIGHTS per MM (col_grp-restricted), and mixed f32r/bf16 operands
        # are rejected by the compiler; the fp32 path sustains ~57ns/MM.
        fcw_t = consts.tile([PIXM, NCHK, NA * NCLS], fc_sdt)
        p_all = consts.tile([PIXM, NCHK, B_LOC, NA], fc_sdt)

        half = NPIX // 2  # 392

        def _ms(ap):
            # memset rejects float32r; zero bits through a float32 view
            nc.gpsimd.memset(ap.bitcast(_F32) if conv_sdt == _F32R else ap, 0.0)

        def conv1(b, x9_t, bi):
            """h(b) = relu(conv1(x(b))) written into a padded 30x30 image."""
            hp = hp_pool.tile([NCH1, HP_LEN], conv_sdt, tag="hp")
            hp3 = hp[:, :PW * PW].rearrange("p (y x) -> p y x", x=PW)
            # zero the 1-pixel border (interior is fully overwritten below)
            _ms(hp3[:, 0, :])
            _ms(hp3[:, PW - 1, :])
            _ms(hp3[:, 1:PW - 1, 0])
            _ms(hp3[:, 1:PW - 1, PW - 1])
            _ms(hp[:, PW * PW:])
            kk = x9_t.shape[0]  # 9 for early chunks, 128 for steady buffers
            for h in range(2):
                ps1 = ps1_pool.tile([NCH1, half], _F32, tag="ps1")
                nc.tensor.matmul(
                    ps1[:],
                    lhsT=_mm(w1c_t[0:kk, :], C1_DT),
                    rhs=_mm(x9_t[:, bi, h * half:(h + 1) * half], C1_DT),
                    start=True, stop=True,
                )
                # relu + downcast into hpad interior rows 14h..14h+13
                dst = hp3[:, 1 + 14 * h:1 + 14 * (h + 1), 1:1 + HW]
                nc.scalar.activation(
                    dst, ps1[:].rearrange("p (y x) -> p y x", x=HW),
                    mybir.ActivationFunctionType.Relu,
                )
            return hp

        def conv2(b, hp):
            """h2(b) -> relu -> group-pool into p_all[:, :, b, :].

            Chunks are processed in pairs sharing one PSUM bank (2 x 256 f32
            = one 2KB bank): the pair's first matmul start=True zeroes the
            whole bank; the second chunk's matmuls rely on pending-zero for
            their first write.  Halves the sem-inc rounds on PE and the
            relu/pool op counts."""
            for ci, cs in enumerate([(0, 1), (2, 3), (4, 5), (6,)]):
                nc2 = len(cs) * NCH2
                ps2 = ps2_pool.tile([PIXM, nc2], _F32, tag="ps2")
                nmm = len(cs) * 9
                i = 0
                for k, c in enumerate(cs):
                    for tap in range(9):
                        dy, dx = tap // 3, tap % 3
                        off = PIXM * c + dy * PW + dx
                        lhsT = hp[:, off:off + PIXM]
                        nc.tensor.matmul(
                            ps2[:, k * NCH2:(k + 1) * NCH2],
                            lhsT=_mm(lhsT, CONV_DT),
                            rhs=_mm(wt_t[:, tap, :], CONV_DT),
                            start=(i == 0), stop=(i == nmm - 1),
                        )
                        i += 1
                # relu2 on the vector engine (same engine as the pool, so no
                # cross-engine relay; measured faster than scalar or
                # alternating-engine variants).
                h2r = h2_pool.tile([PIXM, nc2], _BF16, tag="h2r")
                nc.vector.tensor_scalar_max(h2r[:], ps2[:], 0.0)
                with nc.allow_low_precision(reason="pool sum feeds reduced-precision fc"):
                    nc.vector.tensor_reduce(
                        p_all[:, cs[0]:cs[0] + len(cs), b, :],
                        h2r[:].rearrange("p (c a k) -> p c a k", k=G, a=NA),
                        axis=mybir.AxisListType.X,
                        op=mybir.AluOpType.add,
                    )

        # software-pipelined main loop (conv1 runs DEPTH images ahead of
        # conv2 so relu/memset latency never stalls the PE); images 0-1 come
        # from the early x9_first chunk
        # Software-pipelined main loop over image PAIRS: conv1 for two images
        # is emitted as one burst so the w1c LDWEIGHTS (row_grp=q0, conflicts
        # with in-flight full-array matmuls and pays a ~95ns pipeline-drain
        # bubble) is paid once per two images instead of once per image.
        # conv1 runs DEPTH=4 images ahead of conv2.
        DEPTH = 4
        hps = {}
        # staggered input chunks: small early chunks land before the compute
        # pipeline drains (a single 8-image first chunk measured a ~1.5us PE
        # stall at startup waiting on its ~multi-us DMA).
        sizes = [2, 2, 4] + [C_IMG] * ((B_LOC - 8) // C_IMG)
        assert sum(sizes) == B_LOC
        x9_t, x0, sz, ci, nbig = x9_first, 0, 2, 1, 0
        for pb in range(0, B_LOC + DEPTH, 2):
            if pb == 2:
                for xt in x9_big:
                    nc.gpsimd.memset(xt[:], 0.0)
            if pb < B_LOC:
                if pb == x0 + sz:
                    x0, sz = pb, sizes[ci]
                    ci += 1
                    if pb in x9_early:
                        x9_t = x9_early.pop(pb)
                    else:
                        x9_t = x9_load(x0, sz, nbig)
                        nbig += 1
                hps[pb] = conv1(pb, x9_t, pb - x0)
                hps[pb + 1] = conv1(pb + 1, x9_t, pb + 1 - x0)
            if pb >= DEPTH:
                conv2(pb - DEPTH, hps.pop(pb - DEPTH))
                conv2(pb - DEPTH + 1, hps.pop(pb - DEPTH + 1))

        nc.sync.dma_start(fcw_t[:], fcw_d[:].rearrange("p (c m) -> p c m", m=NA * NCLS))

        # fc: out[n, bb] = fcb[n] + sum_ca fcw[:, c, a*10+n].T @ p_all[:, c, :, a]
        # The bias enters as a K=1 rank-one matmul (fcb x ones) opening the
        # accumulation, and the result DMAs straight out of PSUM — no separate
        # vector bias-add or SBUF staging on the critical tail.
        fc_ps = psfc_pool.tile([NCLS, B_LOC], _F32)
        nc.tensor.matmul(fc_ps[:], lhsT=fcb_t[:], rhs=ones_t[:],
                         start=True, stop=False)
        nmm = NCHK * NA
        i = 0
        for c in range(NCHK):
            for a in range(NA):
                nc.tensor.matmul(
                    fc_ps[:],
                    lhsT=_mm(fcw_t[:, c, a * NCLS:(a + 1) * NCLS], FC_DT),
                    rhs=_mm(p_all[:, c, :, a], FC_DT),
                    start=False, stop=(i == nmm - 1),
                )
                i += 1
        # DMA cannot read PSUM; stage through SBUF on the scalar engine,
        # which is idle at the tail (vector is still draining pools).
        out_sb = consts.tile([NCLS, B_LOC], _F32)
        nc.scalar.activation(out_sb[:], fc_ps[:],
                             mybir.ActivationFunctionType.Copy)
        nc.sync.dma_start(out_d[:], out_sb[:])

    if not nc.is_finalized():
        nc.finalize()
    return nc


_NC_CACHE = {}


def _get_nc():
    key = (CONV_DT, FC_DT)
    if key not in _NC_CACHE:
        _NC_CACHE[key] = build_bass()
    return _NC_CACHE[key]


def _run(x, base_weight, w2, fc_w, fc_b, **spmd_kwargs):
    x = np.asarray(x, np.float32)
    base_weight = np.asarray(base_weight, np.float32)
    w2 = np.asarray(w2, np.float32)
    fc_w = np.asarray(fc_w, np.float32)
    fc_b = np.asarray(fc_b, np.float32)

    prep = _host_prep(x, base_weight, w2, fc_w, fc_b)
    nc = _get_nc()
    in_maps = []
    for i in range(N_CORES):
        m = dict(prep)
        m["x9"] = np.ascontiguousarray(prep["x9"][:, i * B_LOC:(i + 1) * B_LOC, :])
        in_maps.append(m)
    res = run_bass_kernel_spmd(nc, in_maps, list(range(N_CORES)), **spmd_kwargs)
    out = np.concatenate(
        [np.ascontiguousarray(res.results[i]["out"].T) for i in range(N_CORES)],
        axis=0)
    return out, res


def kernel(x, base_weight, w2, fc_w, fc_b):
    out, _ = _run(x, base_weight, w2, fc_w, fc_b)
    return out



# revision 44
# speedup vs baseline: 1.0143x; 1.0019x over previous
"""Trainium2 Bass kernel for a steerable group-CNN (GCNN) forward pass.

Pipeline (per image):
  conv1: 1->128 ch, 3x3, pad 1   (rotated-kernel construction done on host)
  relu
  conv2: 128->256 ch, 3x3, pad 1 (circulant group weight, built on host)
  relu
  group-pool: mean over inner-8 channel factor -> 32 ch
  fc: (32*28*28) -> 10

Device strategy (pure data parallel, batch 512 / 8 cores = 64 images/core):
  - conv1 as one matmul per half image (im2col of x built on host):
      out[oc, pix] = sum_tap w1c[tap, oc] * x9[tap, pix]
    The K dim is zero-padded 9->128 (zero weight rows x zeroed input rows)
    so the matmuls are full-array and their weight loads pipeline through
    the PE background buffer with no row_grp drain bubbles.  ReLU'd into a
    zero-padded 30x30 SBUF image (hpad) so conv2 can read shifted windows.
  - conv2 with *shifted activations as the stationary operand*:
      out[(y,x), oc] += hpad[:, y+dy, x+dx].T @ wt[:, (dy,dx), :]
    9 accumulating bf16 matmuls per 128-position chunk (N=256 channels,
    ~107ns each = the PE stream floor; ~85% of the machine's useful-MAC
    peak for this layer).  Output lands pixels-on-partitions /
    channels-free, so the group-pool is a free-dim strided reduce.
  - relu2 + pool both on VectorE (no cross-engine relay); relu1 on ScalarE.
  - pool+fc folded: p = reduce_add over inner-8 channels; the 1/8 mean and
    the flatten order are folded into a host-rearranged fc weight.  FC is a
    pixel-contraction f32r matmul accumulated over (chunk, group) into one
    PSUM, opened by a K=1 rank-one bias matmul; the [classes, batch] result
    DMAs out via a scalar copy and the host transposes.
  - main loop runs over image pairs, conv1 4 images ahead of conv2;
    input chunks are staggered (2,2,4,8,...) so early DMAs land in time.
"""

import os

import numpy as np

import concourse.tile as tile
from concourse import bacc, mybir
from concourse.bass_utils import run_bass_kernel_spmd

G = 8
KS = 3
HW = 28
PW = HW + 2          # padded image width
NPIX = HW * HW       # 784
NCH1 = 128           # conv1 out channels (G*16)
NCH2 = 256           # conv2 out channels (G*32)
NA = 32              # pooled channels
NCLS = 10
# conv2 processes M=128 contiguous *padded* (30-wide) flat positions per chunk:
# chunk c covers padded-flat positions f = 128c + m.  Valid output pixels are
# f = y*30+x with x<28, y<28; everything else is a junk partition annihilated
# by zero rows in the host-built fc weight.
PIXM = 128
NCHK = -(-(HW * PW - 2) // PIXM)   # 7 chunks cover flat [0, 838)
# taps read hp[128c + m + dy*30 + dx]; max = 128*6 + 127 + 62 = 957
HP_LEN = 960
N_CORES = 8
B_TOT = 512
B_LOC = B_TOT // N_CORES      # 64
C_IMG = 8                     # images per x9 DMA chunk

# matmul operand dtype for conv2 (h storage + group weight): "f32", "f32r", "bf16"
CONV_DT = os.environ.get("GCNN_CONV_DT", "bf16")
# conv1 operand dtype (x im2col + rotated base weight).  f32r moving operands
# stream at 2 cycles/column (4-byte reads), so bf16 halves conv1 PE time and
# the x9 DMA bytes; end-to-end rel_fro stays ~4e-3.
C1_DT = os.environ.get("GCNN_C1_DT", "bf16")
# fc/pool dtype: "f32", "f32r", or "bf16".  f32r is fastest here: the fc is
# weight-load-bound (224 tiny-M matmuls), and the fp32 weight path pipelines
# loads at ~57ns/MM while bf16's separate LDWEIGHTS (col_grp-restricted,
# ~104ns, unhidden behind a 27ns matmul) serializes at ~110ns/MM.
FC_DT = os.environ.get("GCNN_FC_DT", "f32r")

_F32 = mybir.dt.float32
_BF16 = mybir.dt.bfloat16
_F32R = mybir.dt.float32r


def _store_dt(kind):
    if kind == "bf16":
        return _BF16
    if kind == "f32r":
        return _F32R
    return _F32


def _np_dt(kind):
    import ml_dtypes
    return ml_dtypes.bfloat16 if kind == "bf16" else np.float32


def _mm(ap, kind):
    return ap


# ---------------------------------------------------------------------------
# Host-side weight construction (replicates the reference's jax math in numpy)
# ---------------------------------------------------------------------------

def _bilinear_sample(img, px, py):
    K = img.shape[-1]
    x0 = np.floor(px)
    y0 = np.floor(py)
    wx = (px - x0).astype(np.float32)
    wy = (py - y0).astype(np.float32)
    x0i = x0.astype(np.int32)
    y0i = y0.astype(np.int32)

    def gather(yi, xi):
        valid = (yi >= 0) & (yi < K) & (xi >= 0) & (xi < K)
        yc = np.clip(yi, 0, K - 1)
        xc = np.clip(xi, 0, K - 1)
        return img[:, :, yc, xc] * valid.astype(img.dtype)

    return (gather(y0i, x0i) * (1 - wx) * (1 - wy)
            + gather(y0i, x0i + 1) * wx * (1 - wy)
            + gather(y0i + 1, x0i) * (1 - wx) * wy
            + gather(y0i + 1, x0i + 1) * wx * wy)


def _rotated_kernels(base, group_order):
    K = base.shape[-1]
    coords = ((2.0 * np.arange(K, dtype=np.float32) + 1.0) / K - 1.0).astype(np.float32)
    xs, ys = np.meshgrid(coords, coords, indexing="xy")
    out = np.empty((group_order,) + base.shape, np.float32)
    for k in range(group_order):
        theta = np.float32(2.0 * np.pi * k / group_order)
        c, s = np.float32(np.cos(theta)), np.float32(np.sin(theta))
        gx = c * xs - s * ys
        gy = s * xs + c * ys
        px = ((gx + 1.0) * K - 1.0) / 2.0
        py = ((gy + 1.0) * K - 1.0) / 2.0
        out[k] = _bilinear_sample(base, px.astype(np.float32), py.astype(np.float32))
    return out


def _host_prep(x, base_weight, w2, fc_w, fc_b):
    conv_np = _np_dt(CONV_DT)
    c1_np = _np_dt("f32r" if C1_DT == "mixed" else C1_DT)
    w1_np = _np_dt("bf16" if C1_DT == "mixed" else C1_DT)
    fc_np = _np_dt(FC_DT)

    rk = _rotated_kernels(base_weight.astype(np.float32), G)   # (G, 16, 1, 3, 3)
    w1 = rk.reshape(G * 16, 1, KS, KS)                         # (128, 1, 3, 3)
    # (128, 128): taps in rows 0-8, rows 9-127 zero.  The zero-padded K=128
    # keeps conv1's matmuls full-array (a K=9 stationary gets row_grp=q0 and
    # its LDWEIGHTS then conflicts with in-flight full-array matmuls, paying
    # a ~95ns pipeline-drain bubble at every conv1 group).
    w1c = np.zeros((NCH1, NCH1), np.float32)
    w1c[:9] = w1[:, 0].reshape(NCH1, 9).T                      # tap=dy*3+dx

    gi = np.arange(G)[:, None]
    hi = np.arange(G)[None, :]
    idx = (gi - hi) % G
    Wc = w2[:, :, idx]                                          # (32, 16, G, G, 3, 3)
    Wbig = np.transpose(Wc, (2, 0, 1, 3, 4, 5)).reshape(NCH2, NCH1, KS, KS)
    # wt[ic, tap, oc] = Wbig[oc, ic, dy, dx]
    wt = np.ascontiguousarray(np.transpose(Wbig, (1, 2, 3, 0))).reshape(NCH1, 9 * NCH2)

    # fcw[m, c, a*10+n] = fc_w[n, a*784 + y*28 + x] / 8 for f = 128c+m = y*30+x
    # when (y, x) is a real pixel; zero for junk positions.
    f8 = (fc_w.astype(np.float64) / 8.0).astype(np.float32).reshape(NCLS, NA, HW, HW)
    fcw = np.zeros((PIXM, NCHK, NA, NCLS), np.float32)
    for c in range(NCHK):
        for m in range(PIXM):
            ff = c * PIXM + m
            yy, xx = ff // PW, ff % PW
            if yy < HW and xx < HW:
                fcw[m, c] = f8[:, :, yy, xx].T
    fcw = np.ascontiguousarray(fcw.reshape(PIXM, NCHK * NA * NCLS))

    # im2col of padded x: x9[tap, b, pix] = xpad[b, y+dy, x+dx]
    B = x.shape[0]
    xp = np.zeros((B, PW, PW), np.float32)
    xp[:, 1:1 + HW, 1:1 + HW] = x[:, 0]
    x9 = np.empty((9, B, HW, HW), np.float32)
    for dy in range(3):
        for dx in range(3):
            x9[dy * 3 + dx] = xp[:, dy:dy + HW, dx:dx + HW]
    x9 = x9.reshape(9, B, NPIX)

    return {
        "x9": np.ascontiguousarray(x9.astype(c1_np)),
        "w1c": np.ascontiguousarray(w1c.astype(w1_np)),
        "wt": np.ascontiguousarray(wt.astype(conv_np)),
        "fcw": np.ascontiguousarray(fcw.astype(fc_np)),
        "fcb": np.ascontiguousarray(fc_b.reshape(1, NCLS).astype(np.float32)),
    }


# ---------------------------------------------------------------------------
# Device kernel
# ---------------------------------------------------------------------------

def build_bass():
    from contextlib import ExitStack

    conv_sdt = _store_dt(CONV_DT)
    c1_sdt = _store_dt("f32r" if C1_DT == "mixed" else C1_DT)
    w1_sdt = _store_dt("bf16" if C1_DT == "mixed" else C1_DT)
    fc_sdt = _store_dt(FC_DT)

    nc = bacc.Bacc()
    x9_d = nc.declare_dram_parameter("x9", [9, B_LOC, NPIX], c1_sdt, isOutput=False)
    w1c_d = nc.declare_dram_parameter("w1c", [NCH1, NCH1], w1_sdt, isOutput=False)
    wt_d = nc.declare_dram_parameter("wt", [NCH1, 9 * NCH2], conv_sdt, isOutput=False)
    fcw_d = nc.declare_dram_parameter("fcw", [PIXM, NCHK * NA * NCLS], fc_sdt,
                                      isOutput=False)
    fcb_d = nc.declare_dram_parameter("fcb", [1, NCLS], _F32, isOutput=False)
    # out is [classes, batch]; the host transposes.  A [batch, classes] DRAM
    # layout would need a `b n -> n b` rearranged DMA = 640 strided 4-byte
    # descriptors (~8 us measured); this way it's 10 contiguous rows.
    out_d = nc.declare_dram_parameter("out", [NCLS, B_LOC], _F32, isOutput=True)

    with tile.TileContext(nc) as tc, ExitStack() as ctx:
        consts = ctx.enter_context(tc.tile_pool(name="consts", bufs=1))
        x9_pool = ctx.enter_context(tc.tile_pool(name="x9", bufs=2))
        hp_pool = ctx.enter_context(tc.tile_pool(name="hpad", bufs=7))
        h2_pool = ctx.enter_context(tc.tile_pool(name="h2r", bufs=4))
        # ps1=4 so a 2-image conv1 burst (4 matmuls) never waits on relu1;
        # ps2=3 is enough because relu2+pool (DVE) trail a chunk-pair by far
        # less than 3 pair-periods.  4+3+1 = 8 PSUM banks.
        ps1_pool = ctx.enter_context(tc.tile_pool(name="ps1", bufs=4, space="PSUM"))
        ps2_pool = ctx.enter_context(tc.tile_pool(name="ps2", bufs=3, space="PSUM"))
        psfc_pool = ctx.enter_context(tc.tile_pool(name="psfc", bufs=1, space="PSUM"))

        # First input chunk: tiny (2 images) and issued before everything else
        # so conv1 can start as early as possible.  Early chunks stay
        # 9-partition (K=9 conv1, row_grp=q0 — bubbles only at startup);
        # steady-state chunks land in two fixed 128-partition buffers whose
        # rows 9-127 are zeroed once, so steady conv1 contracts K=128 with
        # full-array untagged matmuls (no LDWEIGHTS drain bubbles).
        x9_first = consts.tile([9, 2, NPIX], c1_sdt)
        nc.sync.dma_start(x9_first[:], x9_d[:, 0:2, :])
        w1c_t = consts.tile([NCH1, NCH1], w1_sdt)
        nc.sync.dma_start(w1c_t[:], w1c_d[:])

        # PE warm-up: dependency-free matmuls keep the tensor engine busy from
        # engine start, flipping the HAM clock gate to 2.4 GHz before the real
        # work arrives and hiding the initial weight/input DMA latency.  Uses
        # ps1-pool tiles so no PSUM bank is held for the rest of the kernel.
        warm_sb = consts.tile([NCH1, 512], conv_sdt)
        nc.vector.memset(warm_sb[:].bitcast(_F32) if conv_sdt == _F32R else warm_sb[:],
                         0.125)
        for _ in range(3):
            warm_ps = ps1_pool.tile([NCH1, half := NPIX // 2], _F32, tag="ps1")
            nc.tensor.matmul(warm_ps[:], lhsT=warm_sb[:, :NCH1],
                             rhs=warm_sb[:, :half], start=True, stop=True)
            nc.tensor.matmul(warm_ps[:], lhsT=warm_sb[:, :NCH1],
                             rhs=warm_sb[:, :half], start=True, stop=True)

        # The first two in-loop x9 chunks are issued BEFORE the big wt
        # transfer: DMA queues serialize, and the ~590KB wt load otherwise
        # delays chunk 2's landing past when conv1(2) needs it (~2us stall).
        # Early chunks are 9-partition tiles (K=9 conv1).
        x9_early = {}
        for xb, xsz in ((2, 2), (4, 4)):
            xt = x9_pool.tile([9, xsz, NPIX], c1_sdt, tag="x9")
            nc.sync.dma_start(xt[:], x9_d[:, xb:xb + xsz, :])
            x9_early[xb] = xt

        # Two fixed 128-partition buffers for the steady 8-image chunks;
        # their memsets (rows 9-127 stay zero forever) are emitted inside the
        # loop at pb==2 so the gpsimd queue serves hp(0..3) borders first.
        x9_big = [consts.tile([NCH1, C_IMG, NPIX], c1_sdt, name=f"x9big{j}")
                  for j in range(2)]

        def x9_load(xb, xsz, bi):
            xt = x9_big[bi % 2]
            nc.sync.dma_start(xt[0:9, 0:xsz], x9_d[:, xb:xb + xsz, :])
            return xt

        # resident tensors
        wt_t = consts.tile([NCH1, 9, NCH2], conv_sdt)
        nc.sync.dma_start(wt_t[:], wt_d[:].rearrange("p (t o) -> p t o", o=NCH2))
        # bias row [1, 10] and a ones row [1, 64] feed the K=1 bias matmul
        fcb_t = consts.tile([1, NCLS], _F32)
        nc.sync.dma_start(fcb_t[:], fcb_d[:])
        ones_t = consts.tile([1, B_LOC], _F32)
        nc.vector.memset(ones_t[:], 1.0)
        # fcw is only needed by the fc tail; load it off the critical start path
        # fc operands stay f32r: bf16 fc matmuls pay a serialized ~104ns
        # LDWEIGHTS per MM (col_grp-restricted), and mixed f32r/bf16 operands
        # are rejected by the compiler; the fp32 path sustains ~57ns/MM.
        fcw_t = consts.tile([PIXM, NCHK, NA * NCLS], fc_sdt)
        p_all = consts.tile([PIXM, NCHK, B_LOC, NA], fc_sdt)

        half = NPIX // 2  # 392

        def _ms(ap):
            # memset rejects float32r; zero bits through a float32 view
            nc.gpsimd.memset(ap.bitcast(_F32) if conv_sdt == _F32R else ap, 0.0)

        def conv1(b, x9_t, bi):
            """h(b) = relu(conv1(x(b))) written into a padded 30x30 image."""
            hp = hp_pool.tile([NCH1, HP_LEN], conv_sdt, tag="hp")
            hp3 = hp[:, :PW * PW].rearrange("p (y x) -> p y x", x=PW)
            # zero the 1-pixel border (interior is fully overwritten below)
            _ms(hp3[:, 0, :])
            _ms(hp3[:, PW - 1, :])
            _ms(hp3[:, 1:PW - 1, 0])
            _ms(hp3[:, 1:PW - 1, PW - 1])
            _ms(hp[:, PW * PW:])
            kk = x9_t.shape[0]  # 9 for early chunks, 128 for steady buffers
            for h in range(2):
                ps1 = ps1_pool.tile([NCH1, half], _F32, tag="ps1")
                nc.tensor.matmul(
                    ps1[:],
                    lhsT=_mm(w1c_t[0:kk, :], C1_DT),
                    rhs=_mm(x9_t[:, bi, h * half:(h + 1) * half], C1_DT),
                    start=True, stop=True,
                )
                # relu + downcast into hpad interior rows 14h..14h+13
                dst = hp3[:, 1 + 14 * h:1 + 14 * (h + 1), 1:1 + HW]
                nc.scalar.activation(
                    dst, ps1[:].rearrange("p (y x) -> p y x", x=HW),
                    mybir.ActivationFunctionType.Relu,
                )
            return hp

        def conv2(b, hp):
            """h2(b) -> relu -> group-pool into p_all[:, :, b, :].

            Chunks are processed in pairs sharing one PSUM bank (2 x 256 f32
            = one 2KB bank): the pair's first matmul start=True zeroes the
            whole bank; the second chunk's matmuls rely on pending-zero for
            their first write.  Halves the sem-inc rounds on PE and the
            relu/pool op counts."""
            for ci, cs in enumerate([(0, 1), (2, 3), (4, 5), (6,)]):
                nc2 = len(cs) * NCH2
                ps2 = ps2_pool.tile([PIXM, nc2], _F32, tag="ps2")
                nmm = len(cs) * 9
                i = 0
                for k, c in enumerate(cs):
                    for tap in range(9):
                        dy, dx = tap // 3, tap % 3
                        off = PIXM * c + dy * PW + dx
                        lhsT = hp[:, off:off + PIXM]
                        nc.tensor.matmul(
                            ps2[:, k * NCH2:(k + 1) * NCH2],
                            lhsT=_mm(lhsT, CONV_DT),
                            rhs=_mm(wt_t[:, tap, :], CONV_DT),
                            start=(i == 0), stop=(i == nmm - 1),
                        )
                        i += 1
                # relu2 on the vector engine (same engine as the pool, so no
                # cross-engine relay; measured faster than scalar or
                # alternating-engine variants).
                h2r = h2_pool.tile([PIXM, nc2], _BF16, tag="h2r")
                nc.vector.tensor_scalar_max(h2r[:], ps2[:], 0.0)
                with nc.allow_low_precision(reason="pool sum feeds reduced-precision fc"):
                    nc.vector.tensor_reduce(
                        p_all[:, cs[0]:cs[0] + len(cs), b, :],
                        h2r[:].rearrange("p (c a k) -> p c a k", k=G, a=NA),
                        axis=mybir.AxisListType.X,
                        op=mybir.AluOpType.add,
                    )

        # software-pipelined main loop (conv1 runs DEPTH images ahead of
        # conv2 so relu/memset latency never stalls the PE); images 0-1 come
        # from the early x9_first chunk
        # Software-pipelined main loop over image PAIRS: conv1 for two images
        # is emitted as one burst so the w1c LDWEIGHTS (row_grp=q0, conflicts
        # with in-flight full-array matmuls and pays a ~95ns pipeline-drain
        # bubble) is paid once per two images instead of once per image.
        # conv1 runs DEPTH=4 images ahead of conv2.
        DEPTH = 4
        hps = {}
        # staggered input chunks: small early chunks land before the compute
        # pipeline drains (a single 8-image first chunk measured a ~1.5us PE
        # stall at startup waiting on its ~multi-us DMA).
        sizes = [2, 2, 4] + [C_IMG] * ((B_LOC - 8) // C_IMG)
        assert sum(sizes) == B_LOC
        x9_t, x0, sz, ci, nbig = x9_first, 0, 2, 1, 0
        for pb in range(0, B_LOC + DEPTH, 2):
            if pb == 2:
                for xt in x9_big:
                    nc.gpsimd.memset(xt[:], 0.0)
            if pb < B_LOC:
                if pb == x0 + sz:
                    x0, sz = pb, sizes[ci]
                    ci += 1
                    if pb in x9_early:
                        x9_t = x9_early.pop(pb)
                    else:
                        x9_t = x9_load(x0, sz, nbig)
                        nbig += 1
                hps[pb] = conv1(pb, x9_t, pb - x0)
                hps[pb + 1] = conv1(pb + 1, x9_t, pb + 1 - x0)
            if pb >= DEPTH:
                conv2(pb - DEPTH, hps.pop(pb - DEPTH))
                conv2(pb - DEPTH + 1, hps.pop(pb - DEPTH + 1))

        nc.sync.dma_start(fcw_t[:], fcw_d[:].rearrange("p (c m) -> p c m", m=NA * NCLS))

        # fc: out[n, bb] = fcb[n] + sum_ca fcw[:, c, a*10+n].T @ p_all[:, c, :, a]
        # The bias enters as a K=1 rank-one matmul (fcb x ones) opening the
        # accumulation, and the result DMAs straight out of PSUM — no separate
        # vector bias-add or SBUF staging on the critical tail.
        fc_ps = psfc_pool.tile([NCLS, B_LOC], _F32)
        nc.tensor.matmul(fc_ps[:], lhsT=fcb_t[:], rhs=ones_t[:],
                         start=True, stop=False)
        nmm = NCHK * NA
        i = 0
        for c in range(NCHK):
            for a in range(NA):
                nc.tensor.matmul(
                    fc_ps[:],
                    lhsT=_mm(fcw_t[:, c, a * NCLS:(a + 1) * NCLS], FC_DT),
                    rhs=_mm(p_all[:, c, :, a], FC_DT),
                    start=False, stop=(i == nmm - 1),
                )
                i += 1
        # DMA cannot read PSUM; stage through SBUF on the scalar engine,
        # which is idle at the tail (vector is still draining pools).
        out_sb = consts.tile([NCLS, B_LOC], _F32)
        nc.scalar.activation(out_sb[:], fc_ps[:],
                             mybir.ActivationFunctionType.Copy)
        nc.sync.dma_start(out_d[:], out_sb[:])

    if not nc.is_finalized():
        nc.finalize()
    return nc


_NC_CACHE = {}


def _get_nc():
    key = (CONV_DT, FC_DT)
    if key not in _NC_CACHE:
        _NC_CACHE[key] = build_bass()
    return _NC_CACHE[key]


def _run(x, base_weight, w2, fc_w, fc_b, **spmd_kwargs):
    x = np.asarray(x, np.float32)
    base_weight = np.asarray(base_weight, np.float32)
    w2 = np.asarray(w2, np.float32)
    fc_w = np.asarray(fc_w, np.float32)
    fc_b = np.asarray(fc_b, np.float32)

    prep = _host_prep(x, base_weight, w2, fc_w, fc_b)
    nc = _get_nc()
    in_maps = []
    for i in range(N_CORES):
        m = dict(prep)
        m["x9"] = np.ascontiguousarray(prep["x9"][:, i * B_LOC:(i + 1) * B_LOC, :])
        in_maps.append(m)
    res = run_bass_kernel_spmd(nc, in_maps, list(range(N_CORES)), **spmd_kwargs)
    out = np.concatenate(
        [np.ascontiguousarray(res.results[i]["out"].T) for i in range(N_CORES)],
        axis=0)
    return out, res


def kernel(x, base_weight, w2, fc_w, fc_b):
    out, _ = _run(x, base_weight, w2, fc_w, fc_b)
    return out



# revision 53
# speedup vs baseline: 1.0263x; 1.0119x over previous
"""Trainium2 Bass kernel for a steerable group-CNN (GCNN) forward pass.

Pipeline (per image):
  conv1: 1->128 ch, 3x3, pad 1   (rotated-kernel construction done on host)
  relu
  conv2: 128->256 ch, 3x3, pad 1 (circulant group weight, built on host)
  relu
  group-pool: mean over inner-8 channel factor -> 32 ch
  fc: (32*28*28) -> 10

Device strategy (pure data parallel, batch 512 / 8 cores = 64 images/core):
  - conv1 as one matmul per half image (im2col of x built on host):
      out[oc, pix] = sum_tap w1c[tap, oc] * x9[tap, pix]
    The K dim is zero-padded 9->128 (zero weight rows x zeroed input rows)
    so the matmuls are full-array and their weight loads pipeline through
    the PE background buffer with no row_grp drain bubbles.  ReLU'd into a
    zero-padded 30x30 SBUF image (hpad) so conv2 can read shifted windows.
  - conv2 with *shifted activations as the stationary operand*:
      out[(y,x), oc] += hpad[:, y+dy, x+dx].T @ wt[:, (dy,dx), :]
    9 accumulating bf16 matmuls per 128-position chunk (N=256 channels,
    ~107ns each = the PE stream floor; ~85% of the machine's useful-MAC
    peak for this layer).  Output lands pixels-on-partitions /
    channels-free, so the group-pool is a free-dim strided reduce.
  - relu2 + pool both on VectorE (no cross-engine relay); relu1 on ScalarE.
  - pool+fc folded: p = reduce_add over inner-8 channels; the 1/8 mean and
    the flatten order are folded into a host-rearranged fc weight.  FC is a
    pixel-contraction f32r matmul accumulated over (chunk, group) into one
    PSUM, opened by a K=1 rank-one bias matmul; the [classes, batch] result
    DMAs out via a scalar copy and the host transposes.
  - main loop runs over image pairs, conv1 4 images ahead of conv2;
    input chunks are staggered (2,2,4,8,...) so early DMAs land in time.
"""

import os

import numpy as np

import concourse.tile as tile
from concourse import bacc, mybir
from concourse.bass_utils import run_bass_kernel_spmd

G = 8
KS = 3
HW = 28
PW = HW + 2          # padded image width
NPIX = HW * HW       # 784
NCH1 = 128           # conv1 out channels (G*16)
NCH2 = 256           # conv2 out channels (G*32)
NA = 32              # pooled channels
NCLS = 10
# conv2 processes M=128 contiguous *padded* (30-wide) flat positions per chunk:
# chunk c covers padded-flat positions f = 128c + m.  Valid output pixels are
# f = y*30+x with x<28, y<28; everything else is a junk partition annihilated
# by zero rows in the host-built fc weight.
PIXM = 128
NCHK = -(-(HW * PW - 2) // PIXM)   # 7 chunks cover flat [0, 838)
# taps read hp[128c + m + dy*30 + dx]; max = 128*6 + 127 + 62 = 957
HP_LEN = 960
N_CORES = 8
B_TOT = 512
B_LOC = B_TOT // N_CORES      # 64
C_IMG = 8                     # images per x9 DMA chunk

# matmul operand dtype for conv2 (h storage + group weight): "f32", "f32r", "bf16"
CONV_DT = os.environ.get("GCNN_CONV_DT", "bf16")
# conv1 operand dtype (x im2col + rotated base weight).  f32r moving operands
# stream at 2 cycles/column (4-byte reads), so bf16 halves conv1 PE time and
# the x9 DMA bytes; end-to-end rel_fro stays ~4e-3.
C1_DT = os.environ.get("GCNN_C1_DT", "bf16")
# fc/pool dtype.  bf16: the fc runs as 56 full-array matmuls (4 pooled-`a`
# slices share one [128,128] stationary, classes padded 10->32 rows) whose
# 256-column bf16 moving streams hit the 107ns/MM floor with LDWEIGHTS fully
# hidden; cross-`a` junk blocks are discarded by a 3-op DVE extraction.
FC_DT = os.environ.get("GCNN_FC_DT", "bf16")

_F32 = mybir.dt.float32
_BF16 = mybir.dt.bfloat16
_F32R = mybir.dt.float32r


def _store_dt(kind):
    if kind == "bf16":
        return _BF16
    if kind == "f32r":
        return _F32R
    return _F32


def _np_dt(kind):
    import ml_dtypes
    return ml_dtypes.bfloat16 if kind == "bf16" else np.float32


def _mm(ap, kind):
    return ap


# ---------------------------------------------------------------------------
# Host-side weight construction (replicates the reference's jax math in numpy)
# ---------------------------------------------------------------------------

def _bilinear_sample(img, px, py):
    K = img.shape[-1]
    x0 = np.floor(px)
    y0 = np.floor(py)
    wx = (px - x0).astype(np.float32)
    wy = (py - y0).astype(np.float32)
    x0i = x0.astype(np.int32)
    y0i = y0.astype(np.int32)

    def gather(yi, xi):
        valid = (yi >= 0) & (yi < K) & (xi >= 0) & (xi < K)
        yc = np.clip(yi, 0, K - 1)
        xc = np.clip(xi, 0, K - 1)
        return img[:, :, yc, xc] * valid.astype(img.dtype)

    return (gather(y0i, x0i) * (1 - wx) * (1 - wy)
            + gather(y0i, x0i + 1) * wx * (1 - wy)
            + gather(y0i + 1, x0i) * (1 - wx) * wy
            + gather(y0i + 1, x0i + 1) * wx * wy)


def _rotated_kernels(base, group_order):
    K = base.shape[-1]
    coords = ((2.0 * np.arange(K, dtype=np.float32) + 1.0) / K - 1.0).astype(np.float32)
    xs, ys = np.meshgrid(coords, coords, indexing="xy")
    out = np.empty((group_order,) + base.shape, np.float32)
    for k in range(group_order):
        theta = np.float32(2.0 * np.pi * k / group_order)
        c, s = np.float32(np.cos(theta)), np.float32(np.sin(theta))
        gx = c * xs - s * ys
        gy = s * xs + c * ys
        px = ((gx + 1.0) * K - 1.0) / 2.0
        py = ((gy + 1.0) * K - 1.0) / 2.0
        out[k] = _bilinear_sample(base, px.astype(np.float32), py.astype(np.float32))
    return out


def _host_prep(x, base_weight, w2, fc_w, fc_b):
    conv_np = _np_dt(CONV_DT)
    c1_np = _np_dt("f32r" if C1_DT == "mixed" else C1_DT)
    w1_np = _np_dt("bf16" if C1_DT == "mixed" else C1_DT)
    fc_np = _np_dt(FC_DT)

    rk = _rotated_kernels(base_weight.astype(np.float32), G)   # (G, 16, 1, 3, 3)
    w1 = rk.reshape(G * 16, 1, KS, KS)                         # (128, 1, 3, 3)
    # (128, 128): taps in rows 0-8, rows 9-127 zero.  The zero-padded K=128
    # keeps conv1's matmuls full-array (a K=9 stationary gets row_grp=q0 and
    # its LDWEIGHTS then conflicts with in-flight full-array matmuls, paying
    # a ~95ns pipeline-drain bubble at every conv1 group).
    w1c = np.zeros((NCH1, NCH1), np.float32)
    w1c[:9] = w1[:, 0].reshape(NCH1, 9).T                      # tap=dy*3+dx

    gi = np.arange(G)[:, None]
    hi = np.arange(G)[None, :]
    idx = (gi - hi) % G
    Wc = w2[:, :, idx]                                          # (32, 16, G, G, 3, 3)
    Wbig = np.transpose(Wc, (2, 0, 1, 3, 4, 5)).reshape(NCH2, NCH1, KS, KS)
    # wt[ic, tap, oc] = Wbig[oc, ic, dy, dx]
    wt = np.ascontiguousarray(np.transpose(Wbig, (1, 2, 3, 0))).reshape(NCH1, 9 * NCH2)

    # fcw[m, c, a*10+n] = fc_w[n, a*784 + y*28 + x] / 8 for f = 128c+m = y*30+x
    # when (y, x) is a real pixel; zero for junk positions.
    f8 = (fc_w.astype(np.float64) / 8.0).astype(np.float32).reshape(NCLS, NA, HW, HW)
    fcw = np.zeros((PIXM, NCHK, NA, NCLS), np.float32)
    for c in range(NCHK):
        for m in range(PIXM):
            ff = c * PIXM + m
            yy, xx = ff // PW, ff % PW
            if yy < HW and xx < HW:
                fcw[m, c] = f8[:, :, yy, xx].T
    # group 4 `a` slices per matmul with classes padded 10->32 rows so each
    # stationary is a full [128, 128] (M=4*32): see the fc section in
    # build_bass.  Layout: fcw_pad[m, c, a, al*?+n] -> [m, (c, a-group, 128)]
    fcw_pad = np.zeros((PIXM, NCHK, NA, 32), np.float32)
    fcw_pad[..., :NCLS] = fcw
    fcw = np.ascontiguousarray(fcw_pad.reshape(PIXM, NCHK * NA * 32))

    # im2col of padded x: x9[tap, b, pix] = xpad[b, y+dy, x+dx]
    B = x.shape[0]
    xp = np.zeros((B, PW, PW), np.float32)
    xp[:, 1:1 + HW, 1:1 + HW] = x[:, 0]
    x9 = np.empty((9, B, HW, HW), np.float32)
    for dy in range(3):
        for dx in range(3):
            x9[dy * 3 + dx] = xp[:, dy:dy + HW, dx:dx + HW]
    x9 = x9.reshape(9, B, NPIX)

    return {
        "x9": np.ascontiguousarray(x9.astype(c1_np)),
        "w1c": np.ascontiguousarray(w1c.astype(w1_np)),
        "wt": np.ascontiguousarray(wt.astype(conv_np)),
        "fcw": np.ascontiguousarray(fcw.astype(fc_np)),
        "fcb": np.ascontiguousarray(np.pad(fc_b.astype(np.float32),
                                           (0, NCH1 - NCLS)).reshape(1, NCH1)),
    }


# ---------------------------------------------------------------------------
# Device kernel
# ---------------------------------------------------------------------------

def build_bass():
    from contextlib import ExitStack

    conv_sdt = _store_dt(CONV_DT)
    c1_sdt = _store_dt("f32r" if C1_DT == "mixed" else C1_DT)
    w1_sdt = _store_dt("bf16" if C1_DT == "mixed" else C1_DT)
    fc_sdt = _store_dt(FC_DT)

    nc = bacc.Bacc()
    x9_d = nc.declare_dram_parameter("x9", [9, B_LOC, NPIX], c1_sdt, isOutput=False)
    w1c_d = nc.declare_dram_parameter("w1c", [NCH1, NCH1], w1_sdt, isOutput=False)
    wt_d = nc.declare_dram_parameter("wt", [NCH1, 9 * NCH2], conv_sdt, isOutput=False)
    fcw_d = nc.declare_dram_parameter("fcw", [PIXM, NCHK * NA * 32], fc_sdt,
                                      isOutput=False)
    fcb_d = nc.declare_dram_parameter("fcb", [1, NCH1], _F32, isOutput=False)
    # out is [classes, batch]; the host transposes.  A [batch, classes] DRAM
    # layout would need a `b n -> n b` rearranged DMA = 640 strided 4-byte
    # descriptors (~8 us measured); this way it's 10 contiguous rows.
    out_d = nc.declare_dram_parameter("out", [NCLS, B_LOC], _F32, isOutput=True)

    with tile.TileContext(nc) as tc, ExitStack() as ctx:
        consts = ctx.enter_context(tc.tile_pool(name="consts", bufs=1))
        x9_pool = ctx.enter_context(tc.tile_pool(name="x9", bufs=2))
        hp_pool = ctx.enter_context(tc.tile_pool(name="hpad", bufs=7))
        h2_pool = ctx.enter_context(tc.tile_pool(name="h2r", bufs=4))
        # ps1=4 so a 2-image conv1 burst (4 matmuls) never waits on relu1;
        # ps2=3 is enough because relu2+pool (DVE) trail a chunk-pair by far
        # less than 3 pair-periods.  4+3+1 = 8 PSUM banks.
        ps1_pool = ctx.enter_context(tc.tile_pool(name="ps1", bufs=4, space="PSUM"))
        ps2_pool = ctx.enter_context(tc.tile_pool(name="ps2", bufs=3, space="PSUM"))
        psfc_pool = ctx.enter_context(tc.tile_pool(name="psfc", bufs=1, space="PSUM"))

        # First input chunk: tiny (2 images) and issued before everything else
        # so conv1 can start as early as possible.  Early chunks stay
        # 9-partition (K=9 conv1, row_grp=q0 — bubbles only at startup);
        # steady-state chunks land in two fixed 128-partition buffers whose
        # rows 9-127 are zeroed once, so steady conv1 contracts K=128 with
        # full-array untagged matmuls (no LDWEIGHTS drain bubbles).
        x9_first = consts.tile([9, 2, NPIX], c1_sdt)
        nc.sync.dma_start(x9_first[:], x9_d[:, 0:2, :])
        w1c_t = consts.tile([NCH1, NCH1], w1_sdt)
        nc.sync.dma_start(w1c_t[:], w1c_d[:])

        # PE warm-up: dependency-free matmuls keep the tensor engine busy from
        # engine start, flipping the HAM clock gate to 2.4 GHz before the real
        # work arrives and hiding the initial weight/input DMA latency.  Uses
        # ps1-pool tiles so no PSUM bank is held for the rest of the kernel.
        warm_sb = consts.tile([NCH1, 512], conv_sdt)
        nc.vector.memset(warm_sb[:].bitcast(_F32) if conv_sdt == _F32R else warm_sb[:],
                         0.125)
        for _ in range(3):
            warm_ps = ps1_pool.tile([NCH1, half := NPIX // 2], _F32, tag="ps1")
            nc.tensor.matmul(warm_ps[:], lhsT=warm_sb[:, :NCH1],
                             rhs=warm_sb[:, :half], start=True, stop=True)
            nc.tensor.matmul(warm_ps[:], lhsT=warm_sb[:, :NCH1],
                             rhs=warm_sb[:, :half], start=True, stop=True)

        # The first two in-loop x9 chunks are issued BEFORE the big wt
        # transfer: DMA queues serialize, and the ~590KB wt load otherwise
        # delays chunk 2's landing past when conv1(2) needs it (~2us stall).
        # Early chunks are 9-partition tiles (K=9 conv1).
        x9_early = {}
        for xb, xsz in ((2, 2), (4, 4)):
            xt = x9_pool.tile([9, xsz, NPIX], c1_sdt, tag="x9")
            nc.sync.dma_start(xt[:], x9_d[:, xb:xb + xsz, :])
            x9_early[xb] = xt

        # Two fixed 128-partition buffers for the steady 8-image chunks;
        # their memsets (rows 9-127 stay zero forever) are emitted inside the
        # loop at pb==2 so the gpsimd queue serves hp(0..3) borders first.
        x9_big = [consts.tile([NCH1, C_IMG, NPIX], c1_sdt, name=f"x9big{j}")
                  for j in range(2)]

        def x9_load(xb, xsz, bi):
            xt = x9_big[bi % 2]
            nc.sync.dma_start(xt[0:9, 0:xsz], x9_d[:, xb:xb + xsz, :])
            return xt

        # resident tensors
        wt_t = consts.tile([NCH1, 9, NCH2], conv_sdt)
        nc.sync.dma_start(wt_t[:], wt_d[:].rearrange("p (t o) -> p t o", o=NCH2))
        # bias row [1, 128] (classes in cols 0-9, rest zero) and a ones row
        # [1, 256] feed the K=1 rank-one bias matmul opening the fc group
        fcb_t = consts.tile([1, NCH1], _F32)
        nc.sync.dma_start(fcb_t[:], fcb_d[:])
        ones_t = consts.tile([1, 4 * B_LOC], _F32)
        nc.vector.memset(ones_t[:], 1.0)
        # fcw is only needed by the fc tail; load it off the critical start
        # path.  Layout [pix, c, a, 32cls-padded]; p_all [pix, c, a, b] so a
        # 4-`a` moving slice is 256 contiguous bf16 columns.
        fcw_t = consts.tile([PIXM, NCHK, NA * 32], fc_sdt)
        p_all = consts.tile([PIXM, NCHK, NA, B_LOC], fc_sdt)

        half = NPIX // 2  # 392

        def _ms(ap):
            # memset rejects float32r; zero bits through a float32 view
            nc.gpsimd.memset(ap.bitcast(_F32) if conv_sdt == _F32R else ap, 0.0)

        def conv1(b, x9_t, bi):
            """h(b) = relu(conv1(x(b))) written into a padded 30x30 image."""
            hp = hp_pool.tile([NCH1, HP_LEN], conv_sdt, tag="hp")
            hp3 = hp[:, :PW * PW].rearrange("p (y x) -> p y x", x=PW)
            # zero the 1-pixel border (interior is fully overwritten below)
            _ms(hp3[:, 0, :])
            _ms(hp3[:, PW - 1, :])
            _ms(hp3[:, 1:PW - 1, 0])
            _ms(hp3[:, 1:PW - 1, PW - 1])
            _ms(hp[:, PW * PW:])
            kk = x9_t.shape[0]  # 9 for early chunks, 128 for steady buffers
            for h in range(2):
                ps1 = ps1_pool.tile([NCH1, half], _F32, tag="ps1")
                nc.tensor.matmul(
                    ps1[:],
                    lhsT=_mm(w1c_t[0:kk, :], C1_DT),
                    rhs=_mm(x9_t[:, bi, h * half:(h + 1) * half], C1_DT),
                    start=True, stop=True,
                )
                # relu + downcast into hpad interior rows 14h..14h+13
                dst = hp3[:, 1 + 14 * h:1 + 14 * (h + 1), 1:1 + HW]
                nc.scalar.activation(
                    dst, ps1[:].rearrange("p (y x) -> p y x", x=HW),
                    mybir.ActivationFunctionType.Relu,
                )
            return hp

        def conv2(b, hp):
            """h2(b) -> relu -> group-pool into p_all[:, :, b, :].

            Chunks are processed in pairs sharing one PSUM bank (2 x 256 f32
            = one 2KB bank): the pair's first matmul start=True zeroes the
            whole bank; the second chunk's matmuls rely on pending-zero for
            their first write.  Halves the sem-inc rounds on PE and the
            relu/pool op counts."""
            for ci, cs in enumerate([(0, 1), (2, 3), (4, 5), (6,)]):
                nc2 = len(cs) * NCH2
                ps2 = ps2_pool.tile([PIXM, nc2], _F32, tag="ps2")
                nmm = len(cs) * 9
                i = 0
                for k, c in enumerate(cs):
                    for tap in range(9):
                        dy, dx = tap // 3, tap % 3
                        off = PIXM * c + dy * PW + dx
                        lhsT = hp[:, off:off + PIXM]
                        nc.tensor.matmul(
                            ps2[:, k * NCH2:(k + 1) * NCH2],
                            lhsT=_mm(lhsT, CONV_DT),
                            rhs=_mm(wt_t[:, tap, :], CONV_DT),
                            start=(i == 0), stop=(i == nmm - 1),
                        )
                        i += 1
                # relu2 on the vector engine (same engine as the pool, so no
                # cross-engine relay; measured faster than scalar or
                # alternating-engine variants).
                h2r = h2_pool.tile([PIXM, nc2], _BF16, tag="h2r")
                nc.vector.tensor_scalar_max(h2r[:], ps2[:], 0.0)
                with nc.allow_low_precision(reason="pool sum feeds reduced-precision fc"):
                    nc.vector.tensor_reduce(
                        p_all[:, cs[0]:cs[0] + len(cs), :, b],
                        h2r[:].rearrange("p (c a k) -> p c a k", k=G, a=NA),
                        axis=mybir.AxisListType.X,
                        op=mybir.AluOpType.add,
                    )

        # software-pipelined main loop (conv1 runs DEPTH images ahead of
        # conv2 so relu/memset latency never stalls the PE); images 0-1 come
        # from the early x9_first chunk
        # Software-pipelined main loop over image PAIRS: conv1 for two images
        # is emitted as one burst so the w1c LDWEIGHTS (row_grp=q0, conflicts
        # with in-flight full-array matmuls and pays a ~95ns pipeline-drain
        # bubble) is paid once per two images instead of once per image.
        # conv1 runs DEPTH=4 images ahead of conv2.
        DEPTH = 4
        hps = {}
        # staggered input chunks: small early chunks land before the compute
        # pipeline drains (a single 8-image first chunk measured a ~1.5us PE
        # stall at startup waiting on its ~multi-us DMA).
        sizes = [2, 2, 4] + [C_IMG] * ((B_LOC - 8) // C_IMG)
        assert sum(sizes) == B_LOC
        x9_t, x0, sz, ci, nbig = x9_first, 0, 2, 1, 0
        for pb in range(0, B_LOC + DEPTH, 2):
            if pb == 2:
                for xt in x9_big:
                    nc.gpsimd.memset(xt[:], 0.0)
            if pb < B_LOC:
                if pb == x0 + sz:
                    x0, sz = pb, sizes[ci]
                    ci += 1
                    if pb in x9_early:
                        x9_t = x9_early.pop(pb)
                    else:
                        x9_t = x9_load(x0, sz, nbig)
                        nbig += 1
                hps[pb] = conv1(pb, x9_t, pb - x0)
                hps[pb + 1] = conv1(pb + 1, x9_t, pb + 1 - x0)
            if pb >= DEPTH:
                conv2(pb - DEPTH, hps.pop(pb - DEPTH))
                conv2(pb - DEPTH + 1, hps.pop(pb - DEPTH + 1))

        nc.sync.dma_start(fcw_t[:], fcw_d[:].rearrange("p (c m) -> p c m", m=NA * 32))

        # fc: out[n, bb] = fcb[n] + sum_{c,a} fcw[:, c, a, n].T @ p_all[:, c, a, :]
        # Grouped: each matmul takes a full [128, 128] stationary holding 4
        # `a`-slices of fcw (classes padded 10->32 rows) against a 256-column
        # moving slice (4 `a` x 64 b).  All 56 matmuls accumulate into one
        # [128, 256] PSUM tile; wanted products live in the 4 diagonal
        # [32-row x 64-col] blocks, cross-`a` junk in the off-diagonal
        # blocks.  M=128 keeps the LDWEIGHTS untagged and fully hidden under
        # the 107ns matmul stream (the old per-(c,a) form was weight-load
        # bound at 57ns x 224).  The bias opens the group as a K=1 rank-one
        # matmul into rows 0-9 (only the a=0 diagonal block reads those).
        fc_ps = psfc_pool.tile([PIXM, 4 * B_LOC], _F32)
        nc.tensor.matmul(fc_ps[:], lhsT=fcb_t[:], rhs=ones_t[:],
                         start=True, stop=False)
        NGRP = NA // 4
        nmm = NCHK * NGRP
        i = 0
        for c in range(NCHK):
            for g in range(NGRP):
                nc.tensor.matmul(
                    fc_ps[:],
                    lhsT=fcw_t[:, c, g * 128:(g + 1) * 128],
                    rhs=p_all[:, c, 4 * g:4 * (g + 1), :],
                    start=False, stop=(i == nmm - 1),
                )
                i += 1
        # diagonal-block extraction (partition bases 0/32/64/96 are the
        # engine-legal quadrant offsets): out = sum of the 4 wanted blocks.
        # tensor_tensor may read at most one PSUM operand, so chain through
        # SBUF: copy block0, then add blocks 1-3 one at a time.
        t0 = consts.tile([NCLS, B_LOC], _F32)
        t1 = consts.tile([NCLS, B_LOC], _F32)
        t2 = consts.tile([NCLS, B_LOC], _F32)
        out_sb = consts.tile([NCLS, B_LOC], _F32)
        nc.scalar.activation(t0[:], fc_ps[0:NCLS, 0:B_LOC],
                             mybir.ActivationFunctionType.Copy)
        nc.vector.tensor_tensor(t1[:], fc_ps[32:32 + NCLS, B_LOC:2 * B_LOC],
                                t0[:], mybir.AluOpType.add)
        nc.vector.tensor_tensor(t2[:], fc_ps[64:64 + NCLS, 2 * B_LOC:3 * B_LOC],
                                t1[:], mybir.AluOpType.add)
        nc.vector.tensor_tensor(out_sb[:], fc_ps[96:96 + NCLS, 3 * B_LOC:4 * B_LOC],
                                t2[:], mybir.AluOpType.add)
        nc.sync.dma_start(out_d[:], out_sb[:])

    if not nc.is_finalized():
        nc.finalize()
    return nc


_NC_CACHE = {}


def _get_nc():
    key = (CONV_DT, FC_DT)
    if key not in _NC_CACHE:
        _NC_CACHE[key] = build_bass()
    return _NC_CACHE[key]


def _run(x, base_weight, w2, fc_w, fc_b, **spmd_kwargs):
    x = np.asarray(x, np.float32)
    base_weight = np.asarray(base_weight, np.float32)
    w2 = np.asarray(w2, np.float32)
    fc_w = np.asarray(fc_w, np.float32)
    fc_b = np.asarray(fc_b, np.float32)

    prep = _host_prep(x, base_weight, w2, fc_w, fc_b)
    nc = _get_nc()
    in_maps = []
    for i in range(N_CORES):
        m = dict(prep)
        m["x9"] = np.ascontiguousarray(prep["x9"][:, i * B_LOC:(i + 1) * B_LOC, :])
        in_maps.append(m)
    res = run_bass_kernel_spmd(nc, in_maps, list(range(N_CORES)), **spmd_kwargs)
    out = np.concatenate(
        [np.ascontiguousarray(res.results[i]["out"].T) for i in range(N_CORES)],
        axis=0)
    return out, res


def kernel(x, base_weight, w2, fc_w, fc_b):
    out, _ = _run(x, base_weight, w2, fc_w, fc_b)
    return out

